# revision 4
# baseline (speedup 1.0000x reference)
"""NumEncoder GNN kernel for 8 Trainium2 NeuronCores.

Data-parallel over the batch dim (B=32 -> 4 samples/core). Full inputs
in, full outputs out; sharding/gather on host.

Key structure per core (4 samples):
- graphs built on device from num_order_pad (compares + ones-matmul rowsums)
- 2 GNN hops, activations kept transposed [D, B*N]; matmuls in bf16
  (weights host-cast), accumulation fp32 in PSUM; LayerNorm per sample
  via bn_stats in normal layout
- streaming pass out = enc with scatter rows host-poisoned to -3e38;
  running max fused (split across DVE and GPSIMD); scatter rows
  (enc[pos] + node, fp32) stored directly and max-merged at the end
- scatter positions are baked into the program at build time from
  num_pos_pad (available before compile)
"""
import numpy as np
import ml_dtypes

import concourse.bass as bass
import concourse.tile as tile
import concourse.mybir as mybir
from concourse.bass_utils import run_bass_kernel_spmd

F32 = mybir.dt.float32
BF16 = mybir.dt.bfloat16
I32 = mybir.dt.int32
AF = mybir.ActivationFunctionType
OP = mybir.AluOpType

B, S, N, D, HOPS = 32, 4096, 64, 512, 2
NCORES = 8
BL = B // NCORES          # 4 samples per core
ROW = BL * D              # 2048 floats per (s) row in the local enc
NEG = -3.0e38
EPS = 1e-5
NCHUNK = S // 128         # 32 stream chunks of 128 s-rows


def split_excess_waits(nc, max_waits=1):
    # This walrus build allows at most one sync wait on most instruction
    # ctrl structs (Drain, Matmult ldweights). Fan excess waits out onto
    # preceding InstEventSemaphore carriers on the same engine.
    for f in nc.m.functions:
        for bb in f.blocks:
            newlist = []
            for ins in bb.instructions:
                si = getattr(ins, "sync_info", None)
                waits = list(si.on_wait) if si and si.on_wait else []
                if len(waits) > max_waits:
                    extra, keep = waits[:-max_waits], waits[-max_waits:]
                    for k, w in enumerate(extra):
                        pre = mybir.InstEventSemaphore(
                            name=f"{ins.name}-wsplit{k}", ins=[], outs=[]
                        )
                        pre.engine = ins.engine
                        pre.sync_info = mybir.SyncInfo(on_wait=[w], on_update=[])
                        newlist.append(pre)
                    ins.sync_info = mybir.SyncInfo(
                        on_wait=keep,
                        on_update=list(si.on_update) if si.on_update else [],
                    )
                newlist.append(ins)
            bb.instructions[:] = newlist


def _runs(vals):
    """Split an int sequence into (start_index, count, stride) runs of
    constant stride. Single elements get stride 0."""
    runs = []
    i, n = 0, len(vals)
    while i < n:
        j = i + 1
        if j < n:
            st = vals[j] - vals[i]
            while j + 1 < n and vals[j + 1] - vals[j] == st:
                j += 1
            runs.append((i, j - i + 1, st))
        else:
            runs.append((i, 1, 0))
        i = j + 1
    return runs


def make_plan(num_pos_pad):
    """Bake the scatter structure for one core's BL samples."""
    samples = []
    for b in range(BL):
        pos = num_pos_pad[b]
        vidx = np.where(pos > -1)[0]
        vpos = pos[vidx]
        assert np.all(np.diff(vpos) > 0), "positions must be strictly ascending"
        samples.append({"vidx": vidx.tolist(), "vpos": vpos.tolist()})
    chunk_rows = [dict() for _ in range(NCHUNK)]
    for b, sm in enumerate(samples):
        for p in sm["vpos"]:
            c, r = p // 128, p % 128
            chunk_rows[c].setdefault(r, set()).add(b)
    shared = all(len(bs) == BL for rows in chunk_rows for bs in rows.values())
    return {"samples": samples, "chunk_rows": chunk_rows, "shared": shared}


def build(plan):
    nc = bass.Bass()
    enc = nc.dram_tensor("enc", [S, BL, D], F32, kind="ExternalInput")
    encpos = nc.dram_tensor("encpos", [BL, N, D], F32, kind="ExternalInput")
    nenc = nc.dram_tensor("nenc", [BL, N, D], F32, kind="ExternalInput")
    order = nc.dram_tensor("order", [BL, N], I32, kind="ExternalInput")
    w1a = nc.dram_tensor("w1a", [HOPS, D, D], BF16, kind="ExternalInput")
    w1b = nc.dram_tensor("w1b", [HOPS, D, D], BF16, kind="ExternalInput")
    w2a = nc.dram_tensor("w2a", [HOPS, D, D], BF16, kind="ExternalInput")
    w2b = nc.dram_tensor("w2b", [HOPS, D, D], BF16, kind="ExternalInput")
    lnsc = nc.dram_tensor("lnsc", [HOPS, D], F32, kind="ExternalInput")
    lnbs = nc.dram_tensor("lnbs", [HOPS, D], F32, kind="ExternalInput")
    eye128 = nc.dram_tensor("eye128", [128, 128], F32, kind="ExternalInput")

    out = nc.dram_tensor("out", [S, BL, D], F32, kind="ExternalOutput")
    numemb = nc.dram_tensor("numemb", [BL, N, D], F32, kind="ExternalOutput")
    po = nc.dram_tensor("po", [BL, D], F32, kind="ExternalOutput")

    smp = plan["samples"]
    chunk_rows = plan["chunk_rows"]
    shared = plan["shared"]

    with tile.TileContext(nc) as tc:
        import contextlib
        with contextlib.ExitStack() as ctx:
            consts = ctx.enter_context(tc.tile_pool(name="consts", bufs=1))
            gpool = ctx.enter_context(tc.tile_pool(name="gnn", bufs=1))
            spool = ctx.enter_context(tc.tile_pool(name="stream", bufs=6))
            ps_d = ctx.enter_context(tc.tile_pool(name="psd", bufs=2, space="PSUM"))
            ps_m = ctx.enter_context(tc.tile_pool(name="psm", bufs=2, space="PSUM"))
            ps_t = ctx.enter_context(tc.tile_pool(name="pst", bufs=2, space="PSUM"))
            ps_s = ctx.enter_context(tc.tile_pool(name="pss", bufs=2, space="PSUM"))

            # ---------- constants ----------
            eye_t = consts.tile([128, 128], F32, tag="eye", name="eye")
            nc.gpsimd.dma_start(eye_t[:], eye128[:, :])
            ones_1p = consts.tile([1, 128], F32, tag="ones1p", name="ones1p")
            nc.vector.memset(ones_1p[:], 1.0)
            ones_col = consts.tile([128, 1], F32, tag="onescol", name="onescol")
            nc.vector.memset(ones_col[:], 1.0)
            eps_t = consts.tile([64, 1], F32, tag="eps", name="eps")
            nc.vector.memset(eps_t[:], EPS)
            ome = consts.tile([64, 64], F32, tag="ome", name="ome")  # 1 - eye
            nc.vector.tensor_scalar(
                ome[:], eye_t[0:64, 0:64], -1.0, 1.0, op0=OP.mult, op1=OP.add
            )

            # weights (bf16): per hop, per dchunk, [128, 512] tiles
            wt = {}
            for nm, drt in (("w1a", w1a), ("w1b", w1b), ("w2a", w2a), ("w2b", w2b)):
                for h in range(HOPS):
                    for dc in range(4):
                        t = consts.tile([128, D], BF16, tag=f"{nm}_{h}_{dc}",
                                        name=f"{nm}_{h}_{dc}")
                        nc.gpsimd.dma_start(t[:], drt[h, dc * 128:(dc + 1) * 128, :])
                        wt[(nm, h, dc)] = t

            # ln scale/bias broadcast to [128, 512]
            lnb = {}
            for nm, drt in (("sc", lnsc), ("bs", lnbs)):
                for h in range(HOPS):
                    row = consts.tile([1, D], F32, tag=f"ln_{nm}_{h}_row",
                                      name=f"ln_{nm}_{h}_row")
                    nc.scalar.dma_start(row[:], bass.AP(drt, h * D, [[1, 1], [1, D]]))
                    ps = ps_s.tile([128, D], F32, tag="small", name="small")
                    nc.tensor.matmul(ps[:], ones_1p[:], row[:], start=True, stop=True)
                    t = consts.tile([128, D], F32, tag=f"ln_{nm}_{h}",
                                    name=f"ln_{nm}_{h}")
                    nc.vector.tensor_copy(t[:], ps[:])
                    lnb[(nm, h)] = t

            # ---------- graph build (fp32 build, bf16 copy for matmul) ----------
            at = {}
            ivr_g = consts.tile([1, BL * N], F32, tag="ivr_g", name="ivr_g")
            ivr_l = consts.tile([1, BL * N], F32, tag="ivr_l", name="ivr_l")
            invrs_row = {"g": ivr_g, "l": ivr_l}
            for b in range(BL):
                oci = gpool.tile([N, 1], I32, tag="oci", name="oci")
                nc.scalar.dma_start(oci[:], bass.AP(order, b * N, [[1, N], [1, 1]]))
                ori = gpool.tile([1, N], I32, tag="ori", name="ori")
                nc.scalar.dma_start(ori[:], bass.AP(order, b * N, [[1, 1], [1, N]]))
                ocol = gpool.tile([N, 1], F32, tag="ocol", name="ocol")
                nc.vector.tensor_copy(ocol[:], oci[:])
                orow = gpool.tile([1, N], F32, tag="orow", name="orow")
                nc.vector.tensor_copy(orow[:], ori[:])

                rps = ps_s.tile([N, N], F32, tag="small", name="small")
                nc.tensor.matmul(rps[:], ones_1p[:1, :N], orow[:], start=True, stop=True)
                R = gpool.tile([N, N], F32, tag="R", name="R")
                nc.vector.tensor_copy(R[:], rps[:])
                mrow = gpool.tile([N, N], F32, tag="mrow", name="mrow")
                nc.vector.tensor_scalar(mrow[:], R[:], 0.0, None, op0=OP.is_gt)
                mcol = gpool.tile([N, 1], F32, tag="mcol", name="mcol")
                nc.vector.tensor_scalar(mcol[:], ocol[:], 0.0, None, op0=OP.is_gt)

                for g, cmp in (("g", OP.is_gt), ("l", OP.is_le)):
                    a = gpool.tile([N, N], F32, tag=f"at_{g}_{b}", name=f"at_{g}_{b}")
                    nc.vector.tensor_scalar(a[:], R[:], ocol[:], None, op0=cmp)
                    nc.vector.tensor_mul(a[:], a[:], mrow[:])
                    nc.vector.tensor_scalar(a[:], a[:], mcol[:], None, op0=OP.mult)
                    nc.vector.tensor_mul(a[:], a[:], ome[:])
                    nc.vector.tensor_add(a[:], a[:], eye_t[0:64, 0:64])
                    ab = consts.tile([N, N], BF16, tag=f"ab_{g}_{b}", name=f"ab_{g}_{b}")
                    nc.scalar.copy(ab[:], a[:])  # exact: entries are 0/1
                    at[(g, b)] = ab
                    rs = ps_s.tile([1, N], F32, tag="small", name="small")
                    nc.tensor.matmul(rs[:], ones_col[0:N, :], a[:], start=True, stop=True)
                    nc.vector.reciprocal(invrs_row[g][0:1, b * N:(b + 1) * N], rs[:])

            invrs_bc = {}
            for g in ("g", "l"):
                ps = ps_s.tile([128, BL * N], F32, tag="small", name="small")
                nc.tensor.matmul(ps[:], ones_1p[:], invrs_row[g][:], start=True, stop=True)
                t = consts.tile([128, BL * N], F32, tag=f"ivb_{g}", name=f"ivb_{g}")
                nc.vector.tensor_copy(t[:], ps[:])
                invrs_bc[g] = t

            # ---------- initial node = nenc ----------
            node = {}
            node_bf = {}
            nenc_t = {}
            for b in range(BL):
                t = consts.tile([N, D], F32, tag=f"nenc_{b}", name=f"nenc_{b}")
                nc.gpsimd.dma_start(t[:], nenc[b, :, :])
                nenc_t[b] = t
                node[(0, b)] = t

            # ---------- hops ----------
            for h in range(HOPS):
                # bf16 copies of node for the graph matmul lhsT
                for b in range(BL):
                    nb = gpool.tile([N, D], BF16, tag=f"nbf_{b}", name=f"nbf_{b}")
                    nc.scalar.copy(nb[:], node[(h, b)][:])
                    node_bf[(h, b)] = nb

                # message passing (transposed): mT[g][dc] = [128, BL*N] bf16
                mT = {}
                for g in ("g", "l"):
                    for dc in range(4):
                        sc = gpool.tile([128, BL * N], BF16, tag=f"mT_{g}_{dc}",
                                        name=f"mT_{g}_{dc}")
                        mT[(g, dc)] = sc
                for b in range(BL):
                    for g in ("g", "l"):
                        for dc in range(4):
                            mp = ps_m.tile([128, N], F32, tag="mm", name="mm")
                            nc.tensor.matmul(
                                mp[:],
                                node_bf[(h, b)][:, dc * 128:(dc + 1) * 128],
                                at[(g, b)][:],
                                start=True, stop=True,
                            )
                            nc.vector.tensor_tensor(
                                mT[(g, dc)][:, b * N:(b + 1) * N],
                                mp[:],
                                invrs_bc[g][:, b * N:(b + 1) * N],
                                op=OP.mult,
                            )

                # dense: info = relu(m @ wA); gate = sigmoid(info @ wB)
                def dense(src_tiles, wname, act, outtag):
                    res = []
                    for ec in range(4):
                        ps = ps_d.tile([128, BL * N], F32, tag="dense", name="dense")
                        for dc in range(4):
                            nc.tensor.matmul(
                                ps[:],
                                wt[(wname, h, dc)][:, ec * 128:(ec + 1) * 128],
                                src_tiles[dc][:],
                                start=(dc == 0), stop=(dc == 3),
                            )
                        t = gpool.tile([128, BL * N], BF16, tag=f"{outtag}_{ec}",
                                       name=f"{outtag}_{ec}")
                        if act == "relu":
                            nc.vector.tensor_relu(t[:], ps[:])
                        else:
                            nc.scalar.activation(t[:], ps[:], AF.Sigmoid)
                        res.append(t)
                    return res

                m1 = [mT[("g", dc)] for dc in range(4)]
                m2 = [mT[("l", dc)] for dc in range(4)]
                info1 = dense(m1, "w1a", "relu", "i1")
                gate1 = dense(info1, "w1b", "sig", "g1")
                info2 = dense(m2, "w2a", "relu", "i2")
                gate2 = dense(info2, "w2b", "sig", "g2")

                # p = gate1*info1 + gate2*info2 (transposed, fp32 out)
                pT = []
                for fc in range(4):
                    p = gpool.tile([128, BL * N], F32, tag=f"p_{fc}", name=f"p_{fc}")
                    nc.vector.tensor_mul(p[:], gate1[fc][:], info1[fc][:])
                    q = gpool.tile([128, BL * N], F32, tag="ptmp", name="ptmp")
                    nc.vector.tensor_mul(q[:], gate2[fc][:], info2[fc][:])
                    nc.vector.tensor_add(p[:], p[:], q[:])
                    pT.append(p)

                # x = node + p^T  (PE transpose pairs of samples), then LN
                xb = {}
                for b in range(BL):
                    x = gpool.tile([N, D], F32, tag=f"x_{b}", name=f"x_{b}")
                    xb[b] = x
                for fc in range(4):
                    for rp in range(BL // 2):
                        tp = ps_t.tile([128, 128], F32, tag="tr", name="tr")
                        nc.tensor.transpose(
                            tp[:], pT[fc][:, rp * 128:(rp + 1) * 128], eye_t[:]
                        )
                        for bh in range(2):
                            b = rp * 2 + bh
                            nc.vector.tensor_tensor(
                                xb[b][:, fc * 128:(fc + 1) * 128],
                                node[(h, b)][:, fc * 128:(fc + 1) * 128],
                                tp[bh * 64:(bh + 1) * 64, :],
                                op=OP.add,
                            )

                for b in range(BL):
                    stats = gpool.tile([N, 6], F32, tag="stats", name="stats")
                    nc.vector.bn_stats(stats[:], xb[b][:])
                    mv = gpool.tile([N, 2], F32, tag="mv", name="mv")
                    nc.vector.bn_aggr(mv[:], stats[:])
                    std = gpool.tile([N, 1], F32, tag="std", name="std")
                    nc.scalar.activation(
                        std[:], mv[:, 1:2], AF.Sqrt, bias=eps_t[:], scale=1.0
                    )
                    rstd = gpool.tile([N, 1], F32, tag="rstd", name="rstd")
                    nc.vector.reciprocal(rstd[:], std[:])
                    nxt = consts.tile([N, D], F32, tag=f"node_{h + 1}_{b}",
                                      name=f"node_{h + 1}_{b}")
                    nc.vector.tensor_scalar(
                        nxt[:], xb[b][:], mv[:, 0:1], rstd[:],
                        op0=OP.subtract, op1=OP.mult,
                    )
                    nc.vector.tensor_mul(nxt[:], nxt[:], lnb[("sc", h)][0:N, :])
                    nc.vector.tensor_add(nxt[:], nxt[:], lnb[("bs", h)][0:N, :])
                    node[(h + 1, b)] = nxt

            # ---------- num_embedding ----------
            for b in range(BL):
                ne = gpool.tile([N, D], F32, tag=f"ne_{b}", name=f"ne_{b}")
                nc.vector.tensor_add(ne[:], node[(HOPS, b)][:], nenc_t[b][:])
                nc.scalar.dma_start(numemb[b, :, :], ne[:])

            # ---------- updated rows: encpos + node -> direct scatter-store ----------
            updated = {}
            for b in range(BL):
                vidx, vpos = smp[b]["vidx"], smp[b]["vpos"]
                V = len(vidx)
                if V == 0:
                    continue
                ep = gpool.tile([N, D], F32, tag=f"ep_{b}", name=f"ep_{b}")
                nc.scalar.dma_start(ep[0:V, :], encpos[b, 0:V, :])
                u = consts.tile([N, D], F32, tag=f"upd_{b}", name=f"upd_{b}")
                for (r0, cnt, st) in _runs(vidx):
                    if (st in (1, 0)) and vidx[r0] == r0:
                        nc.vector.tensor_tensor(
                            u[r0:r0 + cnt, :], ep[r0:r0 + cnt, :],
                            node[(HOPS, b)][r0:r0 + cnt, :], op=OP.add,
                        )
                    else:
                        stg = gpool.tile([N, D], F32, tag=f"stg_{b}", name=f"stg_{b}")
                        for k in range(cnt):
                            nc.scalar.dma_start(
                                stg[r0 + k:r0 + k + 1, :],
                                node[(HOPS, b)][vidx[r0 + k]:vidx[r0 + k] + 1, :],
                            )
                        nc.vector.tensor_tensor(
                            u[r0:r0 + cnt, :], ep[r0:r0 + cnt, :],
                            stg[r0:r0 + cnt, :], op=OP.add,
                        )
                for (r0, cnt, st) in _runs(vpos):
                    stride = st if cnt > 1 else 1
                    nc.scalar.dma_start(
                        bass.AP(out, vpos[r0] * ROW + b * D,
                                [[stride * ROW, cnt], [1, D]]),
                        u[r0:r0 + cnt, :],
                    )
                updated[b] = (u, V)

            # ---------- stream: out = enc (poisoned rows excluded), running max ----
            accA = consts.tile([128, ROW], F32, tag="accA", name="accA")
            nc.vector.memset(accA[:], NEG)
            for c in range(NCHUNK):
                t = spool.tile([128, ROW], F32, tag="chunk", name="chunk")
                nc.gpsimd.dma_start(t[:], enc[c * 128:(c + 1) * 128, :, :])
                rows = sorted(chunk_rows[c].keys())
                nc.vector.tensor_max(accA[:], accA[:], t[:])
                # stores exclude scatter (poisoned) rows
                if shared:
                    lo = 0
                    for r in rows + [128]:
                        if r > lo:
                            nc.sync.dma_start(
                                out[c * 128 + lo:c * 128 + r, :, :], t[lo:r, :]
                            )
                        lo = r + 1
                else:
                    for b in range(BL):
                        brows = sorted(r for r, bs in chunk_rows[c].items() if b in bs)
                        lo = 0
                        for r in brows + [128]:
                            if r > lo:
                                nc.sync.dma_start(
                                    bass.AP(out, (c * 128 + lo) * ROW + b * D,
                                            [[ROW, r - lo], [1, D]]),
                                    t[lo:r, b * D:(b + 1) * D],
                                )
                            lo = r + 1

            # merge the updated rows into accA
            for b, (u, V) in updated.items():
                nc.vector.tensor_tensor(
                    accA[0:V, b * D:(b + 1) * D],
                    accA[0:V, b * D:(b + 1) * D],
                    u[0:V, :], op=OP.max,
                )

            # ---------- problem_output: fold accA over partitions ----------
            po_sb = consts.tile([128, BL * 4], F32, tag="po", name="po")
            for b in range(BL):
                for q in range(4):
                    tp = ps_t.tile([128, 128], F32, tag="tr", name="tr")
                    nc.tensor.transpose(
                        tp[:], accA[:, b * D + q * 128:b * D + (q + 1) * 128],
                        eye_t[:],
                    )
                    nc.vector.reduce_max(
                        po_sb[:, b * 4 + q:b * 4 + q + 1], tp[:],
                        axis=mybir.AxisListType.X,
                    )
            nc.scalar.dma_start(
                bass.AP(po, 0, [[1, 128], [D, BL], [128, 4]]), po_sb[:]
            )

    split_excess_waits(nc)
    return nc


def _prep_in_maps(inputs):
    enc = np.ascontiguousarray(inputs["encoder_outputs"], dtype=np.float32)
    nenc = np.ascontiguousarray(inputs["num_encoder_outputs"], dtype=np.float32)
    order = np.ascontiguousarray(inputs["num_order_pad"].astype(np.int32))
    pos = np.asarray(inputs["num_pos_pad"])
    consts = {
        "w1a": np.ascontiguousarray(inputs["w1a"]).astype(ml_dtypes.bfloat16),
        "w1b": np.ascontiguousarray(inputs["w1b"]).astype(ml_dtypes.bfloat16),
        "w2a": np.ascontiguousarray(inputs["w2a"]).astype(ml_dtypes.bfloat16),
        "w2b": np.ascontiguousarray(inputs["w2b"]).astype(ml_dtypes.bfloat16),
        "lnsc": np.ascontiguousarray(inputs["ln_scale"], dtype=np.float32),
        "lnbs": np.ascontiguousarray(inputs["ln_bias"], dtype=np.float32),
        "eye128": np.eye(128, dtype=np.float32),
    }
    in_maps, plans = [], []
    for c in range(NCORES):
        sl = slice(c * BL, (c + 1) * BL)
        enc_l = np.ascontiguousarray(enc[:, sl, :])
        plan = make_plan(pos[sl])
        # extract scatter rows, then poison them with -inf so the stream
        # max can run over whole tiles
        encpos = np.zeros((BL, N, D), dtype=np.float32)
        for b, sm in enumerate(plan["samples"]):
            vp = sm["vpos"]
            if vp:
                encpos[b, :len(vp)] = enc_l[vp, b, :]
                enc_l[vp, b, :] = NEG
        in_maps.append({
            "enc": enc_l,
            "encpos": encpos,
            "nenc": np.ascontiguousarray(nenc[sl]),
            "order": order[sl],
            **consts,
        })
        plans.append(plan)
    return in_maps, plans


def _assemble(results):
    gnn = np.concatenate([r["out"] for r in results], axis=1)
    ne = np.concatenate([r["numemb"] for r in results], axis=0)
    po = np.concatenate([r["po"] for r in results], axis=0)
    return gnn, ne, po


def run(inputs, trace=False):
    in_maps, plans = _prep_in_maps(inputs)
    # SPMD: one program for all cores that share a scatter structure
    # (always true for this generator: positions broadcast across batch).
    groups = {}
    for c, p in enumerate(plans):
        key = repr(p["samples"])
        groups.setdefault(key, []).append(c)
    results = [None] * NCORES
    exec_ns = None
    for key, cores in groups.items():
        nc = build(plans[cores[0]])
        res = run_bass_kernel_spmd(
            nc, [in_maps[c] for c in cores], core_ids=cores, trace=trace
        )
        for i, c in enumerate(cores):
            results[c] = res.results[i]
        if res.exec_time_ns is not None:
            exec_ns = max(exec_ns or 0, res.exec_time_ns)
    return _assemble(results), exec_ns


def kernel(**inputs):
    (gnn, ne, po), _ = run(inputs, trace=False)
    return gnn, ne, po
